# revision 1
# baseline (speedup 1.0000x reference)
"""GATv2 (3-layer, residual) Trainium2 kernel — 8-core SPMD, v3.

v3 = v2's slab-batched edge phase + single-tile scatter cells (128-wide
one-hot; matmul cost scales with output width) + (c,h)-major feature layout
so the per-edge softmax-weight broadcast multiply runs in DVE fast mode.

 - Nodes dealt round-robin: node n -> core n%8, slot n//8.
 - Feature order on device is (c,h): device feature f = c*H + h for original
   (h,c). All weights/att/bias/rmap/msum are permuted host-side to match.
 - Per layer: xl = h @ W per 128-row tile (TensorE), streamed to xl_own,
   AllGather -> xl_full.
 - Edge phase over groups of consecutive dst tiles (budgeted chunks):
   dma_gather xj (4 subtable calls) + xi (1 call, from xl_own) into slabs;
   slab DVE/ACT: ea=lrelu(xi+xj), ea*=att, alpha=reduce_c, ex=exp(alpha),
   xj*=ex; per chunk: one-hot Pm via is_equal, po[fs]/pden scatter matmuls
   accumulated per dst tile; per-tile tail: normalize, residual+elu -> hT
   (l<2) or head-mean + bias -> out rows (l=2).
"""

import sys

sys.path.insert(0, "/opt/trn_rl_repo")

import numpy as np
import ml_dtypes

import concourse.bacc as bacc
import concourse.bass as bass
import concourse.tile as tile
from concourse import mybir
from concourse import bass_utils
from concourse.masks import make_identity

BF16 = mybir.dt.bfloat16
F32 = mybir.dt.float32
I16 = mybir.dt.int16
AL = mybir.AluOpType
AF = mybir.ActivationFunctionType
AX = mybir.AxisListType

NCORES = 8
P = 128
HEADS = 4
NSUB = 4
NEG_SLOPE = 0.2
RES_ALPHA = 0.1
EPS = 1e-16

nbf = ml_dtypes.bfloat16


# --------------------------------------------------------------------------
# Host-side preprocessing
# --------------------------------------------------------------------------

def _prep(edge_index, N, budget):
    src = np.asarray(edge_index[0], dtype=np.int64)
    dst = np.asarray(edge_index[1], dtype=np.int64)
    E = src.shape[0]

    S = ((N + NCORES - 1) // NCORES + P - 1) // P * P
    NB = S // P
    SUB = 2 * S
    assert SUB <= 32768

    core_of = dst % NCORES
    slot_of = dst // NCORES          # row in own xl (< S, int16-safe)
    tile_of = slot_of // P
    dstl_of = slot_of % P
    srow = (src % NCORES) * S + src // NCORES
    sub_of = srow // SUB
    sidx = (srow - sub_of * SUB).astype(np.int16)

    cell = ((core_of * NB + tile_of) * NSUB + sub_of).astype(np.int64)
    ncell = NCORES * NB * NSUB
    counts = np.bincount(cell, minlength=ncell).reshape(NCORES, NB, NSUB)
    cnt_chunks = np.ceil(counts / P).astype(np.int64).max(axis=0)  # [NB, NSUB]

    per_tile = cnt_chunks.sum(axis=1)
    budget = max(budget, int(per_tile.max()))

    groups = []
    cur, tot = [], 0
    for k in range(NB):
        c = int(per_tile[k])
        if cur and tot + c > budget:
            groups.append(cur)
            cur, tot = [], 0
        cur.append(k)
        tot += c
    if cur:
        groups.append(cur)

    # chunk numbering in (group, subtable, tile, j) order
    base = np.zeros((NB, NSUB), dtype=np.int64)
    group_info = []
    nch = 0
    for grp in groups:
        g0 = nch
        calls = []
        for t in range(NSUB):
            t0 = nch
            for k in grp:
                base[k, t] = nch
                nch += int(cnt_chunks[k, t])
            if nch > t0:
                calls.append((t, t0, nch - t0))
        group_info.append(dict(tiles=grp, ch0=g0, nch=nch - g0, calls=calls))

    # sort by (cell, src row): edge order within a cell is free, and
    # ascending source rows make the xj dma_gather's HBM reads
    # quasi-sequential instead of random
    eorder = np.argsort(cell * (1 << 17) + srow, kind="stable")
    cnts = np.bincount(cell, minlength=ncell)
    offs = np.concatenate([[0], np.cumsum(cnts)])
    pos_in_cell = np.arange(E, dtype=np.int64) - offs[cell[eorder]]
    e_core = cell[eorder] // (NB * NSUB)
    e_kt = cell[eorder] % (NB * NSUB)
    e_chunk = base.reshape(-1)[e_kt] + pos_in_cell // P
    e_pos = pos_in_cell % P

    idxj = np.zeros((NCORES, nch, P), dtype=np.int16)
    idxi = np.zeros((NCORES, nch, P), dtype=np.int16)
    dstl = np.full((NCORES, P, nch), -1.0, dtype=np.float32)
    idxj[e_core, e_chunk, e_pos] = sidx[eorder]
    idxi[e_core, e_chunk, e_pos] = slot_of[eorder].astype(np.int16)
    dstl[e_core, e_pos, e_chunk] = dstl_of[eorder].astype(np.float32)

    return dict(S=S, NB=NB, SUB=SUB, nch=nch, cnt_chunks=cnt_chunks,
                base=base, groups=group_info, budget=budget,
                idxj=idxj, idxi=idxi, dstl=dstl)


def _wrap_idx_cols(idx_core):
    """[nch, 128] -> [16, nch*8] (the gather's wrapped idx layout)."""
    nch = idx_core.shape[0]
    a = idx_core.reshape(nch, 8, 16)
    return np.ascontiguousarray(
        np.transpose(a, (2, 0, 1)).reshape(16, nch * 8))


def _chperm(F):
    """original feature index (h-major) for each device feature (c-major)."""
    C = F // HEADS
    f = np.arange(F)
    c, h = f // HEADS, f % HEADS
    return h * C + c


# --------------------------------------------------------------------------
# Device program
# --------------------------------------------------------------------------

def build_program(S, NB, SUB, nch, cnt_chunks, base, groups, budget,
                  layers=3, repeat=1, sim=False, lrelu=False, deep=False):
    KIN = 2                       # IN_C = 256
    HC = 128
    F2 = 256
    layer_F = [HC, HC, F2]

    nc = bacc.Bacc("TRN2", target_bir_lowering=False, debug=False,
                   num_devices=NCORES)

    xt_d = [nc.dram_tensor(f"xt{i}", [P, S], BF16, kind="ExternalInput").ap()
            for i in range(KIN)]
    w0_d = [nc.dram_tensor(f"w0_{i}", [P, HC], BF16, kind="ExternalInput").ap()
            for i in range(KIN)]
    w1_d = nc.dram_tensor("w1", [P, HC], BF16, kind="ExternalInput").ap()
    w2_d = nc.dram_tensor("w2", [P, F2], BF16, kind="ExternalInput").ap()
    att_d = [nc.dram_tensor(f"att{i}", [P, layer_F[i]], BF16,
                            kind="ExternalInput").ap() for i in range(3)]
    b01_d = [nc.dram_tensor(f"b{i}", [P, 1], F32, kind="ExternalInput").ap()
             for i in range(2)]
    b2_d = nc.dram_tensor("b2", [64, 1], F32, kind="ExternalInput").ap()
    iota_d = nc.dram_tensor("iota", [P, P], BF16, kind="ExternalInput").ap()
    rmap_d = nc.dram_tensor("rmap", [HEADS, P], F32, kind="ExternalInput").ap()
    msum_d = [nc.dram_tensor(f"msum{i}", [P, 64], BF16,
                             kind="ExternalInput").ap() for i in range(2)]
    idxj_d = nc.dram_tensor("idxj", [16, nch * 8], I16,
                            kind="ExternalInput").ap()
    idxi_d = nc.dram_tensor("idxi", [16, nch * 8], I16,
                            kind="ExternalInput").ap()
    dstl_d = nc.dram_tensor("dstl", [P, nch], BF16, kind="ExternalInput").ap()
    out_d = nc.dram_tensor("out", [S, 64], F32, kind="ExternalOutput").ap()

    with tile.TileContext(nc) as tc:
        with tc.tile_pool(name="const", bufs=1) as cp, \
             tc.tile_pool(name="pers", bufs=1) as pp, \
             tc.tile_pool(name="stageA", bufs=3) as wp, \
             tc.tile_pool(name="slabj", bufs=2) as xjp, \
             tc.tile_pool(name="slabi", bufs=2) as xip, \
             tc.tile_pool(name="smalls", bufs=3 if deep else 2) as sp, \
             tc.tile_pool(name="pm", bufs=5 if deep else 3) as zp, \
             tc.tile_pool(name="tails", bufs=3 if deep else 2) as tp, \
             tc.tile_pool(name="psA", bufs=1 if deep else 2,
                          space="PSUM") as psA, \
             tc.tile_pool(name="psO", bufs=2, space="PSUM") as psO, \
             tc.tile_pool(name="psD", bufs=2 if deep else 1,
                          space="PSUM") as psD, \
             tc.tile_pool(name="psM", bufs=1, space="PSUM") as psM, \
             tc.tile_pool(name="dram", bufs=1, space="DRAM") as dp:

            # ---- constants ----
            w0_sb = [cp.tile([P, HC], BF16, tag=f"w0_{i}", name=f"w0s{i}")
                     for i in range(KIN)]
            for i in range(KIN):
                nc.sync.dma_start(out=w0_sb[i][:], in_=w0_d[i][:])
            w1_sb = cp.tile([P, HC], BF16, tag="w1")
            nc.sync.dma_start(out=w1_sb[:], in_=w1_d[:])
            w2_sb = cp.tile([P, F2], BF16, tag="w2")
            nc.sync.dma_start(out=w2_sb[:], in_=w2_d[:])
            att_sb = []
            for i in range(3):
                t = cp.tile([P, layer_F[i]], BF16, tag=f"att{i}",
                            name=f"atts{i}")
                nc.sync.dma_start(out=t[:], in_=att_d[i][:])
                att_sb.append(t)
            b01_sb = []
            for i in range(2):
                t = cp.tile([P, 1], F32, tag=f"b{i}", name=f"bs{i}")
                nc.sync.dma_start(out=t[:], in_=b01_d[i][:])
                b01_sb.append(t)
            b2_sb = cp.tile([64, 1], F32, tag="b2")
            nc.sync.dma_start(out=b2_sb[:], in_=b2_d[:])
            iota_sb = cp.tile([P, P], BF16, tag="iota")
            nc.sync.dma_start(out=iota_sb[:], in_=iota_d[:])
            rmap_sb = cp.tile([HEADS, P], F32, tag="rmap")
            nc.sync.dma_start(out=rmap_sb[:], in_=rmap_d[:])
            msum_sb = []
            for i in range(2):
                t = cp.tile([P, 64], BF16, tag=f"msum{i}", name=f"msums{i}")
                nc.sync.dma_start(out=t[:], in_=msum_d[i][:])
                msum_sb.append(t)
            dstl_bf = cp.tile([P, nch], BF16, tag="dstlb")
            nc.sync.dma_start(out=dstl_bf[:], in_=dstl_d[:])
            dstl_sb = cp.tile([P, nch], F32, tag="dstl")
            nc.scalar.copy(dstl_sb[:], dstl_bf[:])
            ident = cp.tile([P, P], BF16, tag="ident")
            make_identity(nc, ident[:])
            identf = cp.tile([P, P], F32, tag="identf")
            make_identity(nc, identf[:])

            # ---- persistent ----
            x0T_sb = pp.tile([P, S], BF16, tag="x0T")
            hT_sb = pp.tile([P, S], BF16, tag="hT")

            xl_own = [dp.tile([S, layer_F[l]], BF16, tag=f"xlo{l}",
                              name=f"xlo{l}") for l in range(3)]
            xl_full = [dp.tile([NCORES * S, layer_F[l]], BF16, tag=f"xlf{l}",
                               name=f"xlf{l}") for l in range(3)]
            idxj_rep = dp.tile([P, nch * 8], I16, tag="ijr", name="ijr")
            idxi_rep = dp.tile([P, nch * 8], I16, tag="iir", name="iir")
            for b in range(8):
                nc.sync.dma_start(out=idxj_rep[16 * b:16 * (b + 1), :],
                                  in_=idxj_d[:, :])
                nc.sync.dma_start(out=idxi_rep[16 * b:16 * (b + 1), :],
                                  in_=idxi_d[:, :])

            for l in [ll for _ in range(repeat) for ll in range(layers)]:
                F = layer_F[l]
                C = F // HEADS
                NFS = F // P

                # ---------- phase A:  xl = h @ W ----------
                for k in range(NB):
                    ps = psA.tile([P, F], F32, tag="psA")
                    if l == 0:
                        for i in range(KIN):
                            xa = wp.tile([P, P], BF16, tag=f"xta{i}",
                                         name=f"xta{i}")
                            nc.sync.dma_start(
                                out=xa[:], in_=xt_d[i][:, k * P:(k + 1) * P])
                            nc.tensor.matmul(ps[:], lhsT=xa[:],
                                             rhs=w0_sb[i][:],
                                             start=(i == 0),
                                             stop=(i == KIN - 1))
                    else:
                        w_sb = w1_sb if l == 1 else w2_sb
                        nc.tensor.matmul(ps[:],
                                         lhsT=hT_sb[:, k * P:(k + 1) * P],
                                         rhs=w_sb[:], start=True, stop=True)
                    stage = wp.tile([P, F], BF16, tag="stage")
                    nc.scalar.copy(stage[:], ps[:])
                    nc.sync.dma_start(out=xl_own[l][k * P:(k + 1) * P, :],
                                      in_=stage[:])
                    if l == 0:
                        ptx = psM.tile([P, P], BF16, tag="psM", name="ptx")
                        nc.tensor.transpose(ptx[:], stage[:], ident[:])
                        nc.vector.tensor_scalar(
                            out=x0T_sb[:, k * P:(k + 1) * P], in0=ptx[:],
                            scalar1=RES_ALPHA, scalar2=None, op0=AL.mult)

                # ---------- AllGather ----------
                if sim:
                    for c in range(NCORES):
                        nc.sync.dma_start(
                            out=xl_full[l][c * S:(c + 1) * S, :],
                            in_=xl_own[l][:, :])
                else:
                    nc.gpsimd.collective_compute(
                        "AllGather", AL.bypass,
                        replica_groups=[list(range(NCORES))],
                        ins=[xl_own[l].opt()],
                        outs=[xl_full[l].opt()],
                    )

                # ---------- phase C: edge groups ----------
                for gi in groups:
                    g0 = gi["ch0"]
                    G = gi["nch"]
                    ij = sp.tile([P, budget * 8], I16, tag="ij")
                    ii = sp.tile([P, budget * 8], I16, tag="ii")
                    nc.sync.dma_start(out=ij[:, :G * 8],
                                      in_=idxj_rep[:, g0 * 8:(g0 + G) * 8])
                    nc.sync.dma_start(out=ii[:, :G * 8],
                                      in_=idxi_rep[:, g0 * 8:(g0 + G) * 8])
                    xj = xjp.tile([P, budget * F], BF16, tag="xj")
                    xi = xip.tile([P, budget * F], BF16, tag="xi")
                    for (t, ch0_t, nch_t) in gi["calls"]:
                        sc0 = ch0_t - g0
                        nc.gpsimd.dma_gather(
                            out_ap=xj[:, sc0 * F:(sc0 + nch_t) * F]
                            .rearrange("p (c f) -> p c f", f=F),
                            in_ap=xl_full[l][t * SUB:(t + 1) * SUB, :],
                            idxs_ap=ij[:, sc0 * 8:(sc0 + nch_t) * 8],
                            num_idxs=nch_t * P, num_idxs_reg=nch_t * P,
                            elem_size=F, single_packet=False)
                    nc.gpsimd.dma_gather(
                        out_ap=xi[:, :G * F].rearrange("p (c f) -> p c f",
                                                       f=F),
                        in_ap=xl_own[l][:, :],
                        idxs_ap=ii[:, :G * 8],
                        num_idxs=G * P, num_idxs_reg=G * P,
                        elem_size=F, single_packet=False)

                    # slab ops
                    ea = xi[:, :G * F]
                    nc.vector.tensor_tensor(out=ea, in0=ea,
                                            in1=xj[:, :G * F], op=AL.add)
                    if lrelu:
                        nc.scalar.activation(ea, ea, AF.Lrelu,
                                             alpha=NEG_SLOPE)
                    else:
                        nc.vector.scalar_tensor_tensor(
                            out=ea, in0=ea, scalar=NEG_SLOPE, in1=ea,
                            op0=AL.mult, op1=AL.max)
                    nc.vector.tensor_tensor(
                        out=ea.rearrange("p (g f) -> p g f", f=F),
                        in0=ea.rearrange("p (g f) -> p g f", f=F),
                        in1=att_sb[l][:].unsqueeze(1).broadcast_to([P, G, F]),
                        op=AL.mult)
                    # (c,h) layout: alpha[p,(g,h)] = sum_c ea[p,g,c,h]
                    # binary tree-sum over c: packed bf16 adds run at the
                    # DVE 2x rate vs 1x for tensor_reduce
                    ea4 = ea.rearrange("p (g c h) -> p g c h", h=HEADS, c=C)
                    w = C // 2
                    while w > 1:
                        nc.vector.tensor_tensor(
                            out=ea4[:, :, 0:w, :], in0=ea4[:, :, 0:w, :],
                            in1=ea4[:, :, w:2 * w, :], op=AL.add)
                        w //= 2
                    al = sp.tile([P, budget * HEADS], BF16, tag="al")
                    nc.vector.tensor_tensor(
                        out=al[:, :G * HEADS].rearrange("p (g h) -> p g h",
                                                        h=HEADS),
                        in0=ea4[:, :, 0, :], in1=ea4[:, :, 1, :], op=AL.add)
                    ex = sp.tile([P, budget * HEADS], BF16, tag="ex")
                    nc.scalar.activation(ex[:, :G * HEADS], al[:, :G * HEADS],
                                         AF.Exp)
                    nc.vector.tensor_tensor(
                        out=xj[:, :G * F].rearrange("p (g c h) -> p g c h",
                                                    h=HEADS, c=C),
                        in0=xj[:, :G * F].rearrange("p (g c h) -> p g c h",
                                                    h=HEADS, c=C),
                        in1=ex[:, :G * HEADS]
                            .rearrange("p (g h) -> p g h", h=HEADS)
                            .unsqueeze(2).broadcast_to([P, G, C, HEADS]),
                        op=AL.mult)

                    # scatter + per-tile tails
                    for k in gi["tiles"]:
                        po = [psO.tile([P, P], F32, tag=f"psO{fs}",
                                       name=f"psO{fs}") for fs in range(NFS)]
                        pden = psD.tile([HEADS, P], F32, tag="psD",
                                        name="pden")
                        tile_chunks = []
                        for t in range(NSUB):
                            for j in range(int(cnt_chunks[k, t])):
                                tile_chunks.append(int(base[k, t]) + j)
                        for ci, ch in enumerate(tile_chunks):
                            first = ci == 0
                            last = ci == len(tile_chunks) - 1
                            sc = ch - g0
                            Pm = zp.tile([P, P], BF16, tag="Pm")
                            nc.vector.tensor_scalar(
                                out=Pm[:], in0=iota_sb[:],
                                scalar1=dstl_sb[:, ch:ch + 1], scalar2=None,
                                op0=AL.is_equal)
                            for fs in range(NFS):
                                nc.tensor.matmul(
                                    po[fs][:],
                                    lhsT=xj[:, sc * F + fs * P:
                                            sc * F + (fs + 1) * P],
                                    rhs=Pm[:], start=first, stop=last)
                            nc.tensor.matmul(
                                pden[:],
                                lhsT=ex[:, sc * HEADS:(sc + 1) * HEADS],
                                rhs=Pm[:], start=first, stop=last)

                        # ---- tile tail ----
                        rec = tp.tile([HEADS, P], F32, tag="rec")
                        nc.vector.tensor_scalar(out=rec[:], in0=pden[:],
                                                scalar1=EPS, scalar2=None,
                                                op0=AL.add)
                        nc.vector.reciprocal(rec[:], rec[:])
                        scale = (1.0 - RES_ALPHA) if l < 2 else (1.0 / HEADS)
                        nc.vector.tensor_scalar(out=rec[:], in0=rec[:],
                                                scalar1=scale, scalar2=None,
                                                op0=AL.mult)
                        cols = slice(k * P, (k + 1) * P)
                        if l < 2:
                            prep = psM.tile([P, P], F32, tag="psM",
                                            name="prep")
                            nc.tensor.matmul(prep[:], lhsT=rmap_sb[:],
                                             rhs=rec[:], start=True,
                                             stop=True)
                            rep = tp.tile([P, P], F32, tag="rep")
                            nc.scalar.copy(rep[:], prep[:])
                            u = tp.tile([P, P], F32, tag="u")
                            nc.vector.tensor_tensor(out=u[:], in0=po[0][:],
                                                    in1=rep[:], op=AL.mult)
                            nc.scalar.activation(u[:], u[:], AF.Identity,
                                                 bias=b01_sb[l][:, 0:1])
                            nc.vector.tensor_tensor(out=u[:], in0=u[:],
                                                    in1=x0T_sb[:, cols],
                                                    op=AL.add)
                            mn = tp.tile([P, P], F32, tag="mn")
                            nc.vector.tensor_scalar(out=mn[:], in0=u[:],
                                                    scalar1=0.0,
                                                    scalar2=None, op0=AL.min)
                            em = tp.tile([P, P], F32, tag="em")
                            nc.scalar.activation(em[:], mn[:], AF.Exp)
                            hh = tp.tile([P, P], F32, tag="hh")
                            nc.vector.scalar_tensor_tensor(
                                out=hh[:], in0=u[:], scalar=0.0, in1=em[:],
                                op0=AL.max, op1=AL.add)
                            nc.vector.tensor_scalar(
                                out=hT_sb[:, cols], in0=hh[:],
                                scalar1=-1.0, scalar2=None, op0=AL.add)
                        else:
                            tsb = []
                            for fs in range(2):
                                prep = psM.tile([P, P], F32, tag="psM",
                                                name="prep")
                                nc.tensor.matmul(prep[:], lhsT=rmap_sb[:],
                                                 rhs=rec[:], start=True,
                                                 stop=True)
                                rep = tp.tile([P, P], F32, tag="rep")
                                nc.scalar.copy(rep[:], prep[:])
                                tt = tp.tile([P, P], BF16, tag=f"t{fs}",
                                             name=f"tsb{fs}")
                                nc.vector.tensor_tensor(out=tt[:],
                                                        in0=po[fs][:],
                                                        in1=rep[:],
                                                        op=AL.mult)
                                tsb.append(tt)
                            pmo = psM.tile([64, P], F32, tag="psM",
                                           name="pmo")
                            nc.tensor.matmul(pmo[:], lhsT=msum_sb[0][:],
                                             rhs=tsb[0][:], start=True,
                                             stop=False)
                            nc.tensor.matmul(pmo[:], lhsT=msum_sb[1][:],
                                             rhs=tsb[1][:], start=False,
                                             stop=True)
                            ob = tp.tile([64, P], F32, tag="ob")
                            nc.scalar.activation(ob[:], pmo[:], AF.Identity,
                                                 bias=b2_sb[:, 0:1])
                            pot = psM.tile([P, 64], F32, tag="psM",
                                           name="pot")
                            nc.tensor.transpose(pot[:], ob[:],
                                                identf[:64, :64])
                            orow = tp.tile([P, 64], F32, tag="orow")
                            nc.scalar.copy(orow[:], pot[:])
                            nc.sync.dma_start(
                                out=out_d[k * P:(k + 1) * P, :],
                                in_=orow[:])

    nc.compile()
    return nc


# --------------------------------------------------------------------------
# kernel() entry
# --------------------------------------------------------------------------

def prepare(x, edge_index, W0, b0, att0, W1, b1, att1, W2, b2, att2,
            _budget=40, _layers=3, _repeat=1, _sim=False, _lrelu=False,
            _deep=False):
    x = np.asarray(x, dtype=np.float32)
    N, IN_C = x.shape
    pr = _prep(edge_index, N, _budget)
    S, NB, SUB, nch = pr["S"], pr["NB"], pr["SUB"], pr["nch"]

    nc = build_program(S, NB, SUB, nch, pr["cnt_chunks"], pr["base"],
                       pr["groups"], pr["budget"],
                       layers=_layers, repeat=_repeat, sim=_sim,
                       lrelu=_lrelu, deep=_deep)

    KIN = IN_C // P
    p128 = _chperm(128)     # device feature -> original feature (F=128)
    p256 = _chperm(256)

    W0p = np.asarray(W0, np.float32)[:, p128]           # cols -> (c,h)
    W1p = np.asarray(W1, np.float32)[p128][:, p128]     # rows from hT, cols
    W2p = np.asarray(W2, np.float32)[p128][:, p256]

    common = {}
    for i in range(KIN):
        common[f"w0_{i}"] = W0p[i * P:(i + 1) * P, :].astype(nbf)
    common["w1"] = W1p.astype(nbf)
    common["w2"] = W2p.astype(nbf)

    def rep_att(att, F):
        flat = np.asarray(att, np.float32).reshape(-1)[_chperm(F)]
        return np.repeat(flat[None, :], P, axis=0).astype(nbf)

    common["att0"] = rep_att(att0, 128)
    common["att1"] = rep_att(att1, 128)
    common["att2"] = rep_att(att2, 256)
    common["b0"] = ((1.0 - RES_ALPHA) * np.asarray(b0, np.float32)[p128]
                    ).reshape(P, 1)
    common["b1"] = ((1.0 - RES_ALPHA) * np.asarray(b1, np.float32)[p128]
                    ).reshape(P, 1)
    common["b2"] = np.asarray(b2, np.float32).reshape(64, 1)
    common["iota"] = np.tile(np.arange(P, dtype=np.float32)[None, :],
                             (P, 1)).astype(nbf)
    # head of device feature f is f % HEADS (same map for all layers)
    r0 = np.zeros((HEADS, P), np.float32)
    for f in range(P):
        r0[f % HEADS, f] = 1.0
    common["rmap"] = r0
    # l2 head-mean: device feature g=fs*128+f maps to out channel g//HEADS
    for fs in range(2):
        m = np.zeros((P, 64), np.float32)
        for f in range(P):
            m[f, (fs * P + f) // HEADS] = 1.0
        common[f"msum{fs}"] = m.astype(nbf)

    in_maps = []
    for c in range(NCORES):
        m = dict(common)
        nodes = np.arange(c, N, NCORES, dtype=np.int64)
        xc = np.zeros((S, IN_C), dtype=np.float32)
        xc[:len(nodes)] = x[nodes]
        xct = np.ascontiguousarray(xc.T).astype(nbf)
        for i in range(KIN):
            m[f"xt{i}"] = xct[i * P:(i + 1) * P, :].copy()
        m["idxj"] = _wrap_idx_cols(pr["idxj"][c])
        m["idxi"] = _wrap_idx_cols(pr["idxi"][c])
        m["dstl"] = pr["dstl"][c].astype(nbf)
        in_maps.append(m)

    def assemble(per_core_out):
        out = np.zeros((N, 64), dtype=np.float32)
        for c in range(NCORES):
            nodes = np.arange(c, N, NCORES, dtype=np.int64)
            out[nodes] = per_core_out[c][:len(nodes)]
        return out

    return nc, in_maps, assemble


def kernel(x, edge_index, W0, b0, att0, W1, b1, att1, W2, b2, att2, **kw):
    nc, in_maps, assemble = prepare(x, edge_index, W0, b0, att0, W1, b1,
                                    att1, W2, b2, att2, **kw)
    # run twice: the very first execution on freshly-initialized devices has
    # been observed (rarely) to race; the second run is authoritative.
    bass_utils.run_bass_kernel_spmd(nc, in_maps,
                                    core_ids=list(range(NCORES)))
    res = bass_utils.run_bass_kernel_spmd(nc, in_maps,
                                          core_ids=list(range(NCORES)))
    return assemble([res.results[c]["out"] for c in range(NCORES)])



# revision 3
# speedup vs baseline: 1.1522x; 1.1522x over previous
"""GATv2 (3-layer, residual) Trainium2 kernel — 8-core SPMD, v3.

v3 = v2's slab-batched edge phase + single-tile scatter cells (128-wide
one-hot; matmul cost scales with output width) + (c,h)-major feature layout
so the per-edge softmax-weight broadcast multiply runs in DVE fast mode.

 - Nodes dealt round-robin: node n -> core n%8, slot n//8.
 - Feature order on device is (c,h): device feature f = c*H + h for original
   (h,c). All weights/att/bias/rmap/msum are permuted host-side to match.
 - Per layer: xl = h @ W per 128-row tile (TensorE), streamed to xl_own,
   AllGather -> xl_full.
 - Edge phase over groups of consecutive dst tiles (budgeted chunks):
   dma_gather xj (4 subtable calls) + xi (1 call, from xl_own) into slabs;
   slab DVE/ACT: ea=lrelu(xi+xj), ea*=att, alpha=reduce_c, ex=exp(alpha),
   xj*=ex; per chunk: one-hot Pm via is_equal, po[fs]/pden scatter matmuls
   accumulated per dst tile; per-tile tail: normalize, residual+elu -> hT
   (l<2) or head-mean + bias -> out rows (l=2).
"""

import sys

sys.path.insert(0, "/opt/trn_rl_repo")

import numpy as np
import ml_dtypes

import concourse.bacc as bacc
import concourse.bass as bass
import concourse.tile as tile
from concourse import mybir
from concourse import bass_utils
from concourse.masks import make_identity

BF16 = mybir.dt.bfloat16
F32 = mybir.dt.float32
I16 = mybir.dt.int16
AL = mybir.AluOpType
AF = mybir.ActivationFunctionType
AX = mybir.AxisListType

NCORES = 8
P = 128
HEADS = 4
NSUB = 4
NEG_SLOPE = 0.2
RES_ALPHA = 0.1
EPS = 1e-16

nbf = ml_dtypes.bfloat16


# --------------------------------------------------------------------------
# Host-side preprocessing
# --------------------------------------------------------------------------

def _prep(edge_index, N, budget):
    src = np.asarray(edge_index[0], dtype=np.int64)
    dst = np.asarray(edge_index[1], dtype=np.int64)
    E = src.shape[0]

    S = ((N + NCORES - 1) // NCORES + P - 1) // P * P
    NB = S // P
    SUB = 2 * S
    assert SUB <= 32768

    core_of = dst % NCORES
    slot_of = dst // NCORES          # row in own xl (< S, int16-safe)
    tile_of = slot_of // P
    dstl_of = slot_of % P
    srow = (src % NCORES) * S + src // NCORES
    sub_of = srow // SUB
    sidx = (srow - sub_of * SUB).astype(np.int16)

    cell = ((core_of * NB + tile_of) * NSUB + sub_of).astype(np.int64)
    ncell = NCORES * NB * NSUB
    counts = np.bincount(cell, minlength=ncell).reshape(NCORES, NB, NSUB)
    cnt_chunks = np.ceil(counts / P).astype(np.int64).max(axis=0)  # [NB, NSUB]

    per_tile = cnt_chunks.sum(axis=1)
    budget = max(budget, int(per_tile.max()))

    groups = []
    cur, tot = [], 0
    for k in range(NB):
        c = int(per_tile[k])
        if cur and tot + c > budget:
            groups.append(cur)
            cur, tot = [], 0
        cur.append(k)
        tot += c
    if cur:
        groups.append(cur)

    # chunk numbering in (group, subtable, tile, j) order
    base = np.zeros((NB, NSUB), dtype=np.int64)
    group_info = []
    nch = 0
    for grp in groups:
        g0 = nch
        calls = []
        for t in range(NSUB):
            t0 = nch
            for k in grp:
                base[k, t] = nch
                nch += int(cnt_chunks[k, t])
            if nch > t0:
                calls.append((t, t0, nch - t0))
        group_info.append(dict(tiles=grp, ch0=g0, nch=nch - g0, calls=calls))

    # sort by (cell, src row): edge order within a cell is free, and
    # ascending source rows make the xj dma_gather's HBM reads
    # quasi-sequential instead of random
    eorder = np.argsort(cell * (1 << 17) + srow, kind="stable")
    cnts = np.bincount(cell, minlength=ncell)
    offs = np.concatenate([[0], np.cumsum(cnts)])
    pos_in_cell = np.arange(E, dtype=np.int64) - offs[cell[eorder]]
    e_core = cell[eorder] // (NB * NSUB)
    e_kt = cell[eorder] % (NB * NSUB)
    e_chunk = base.reshape(-1)[e_kt] + pos_in_cell // P
    e_pos = pos_in_cell % P

    idxj = np.zeros((NCORES, nch, P), dtype=np.int16)
    idxi = np.zeros((NCORES, nch, P), dtype=np.int16)
    dstl = np.full((NCORES, P, nch), -1.0, dtype=np.float32)
    idxj[e_core, e_chunk, e_pos] = sidx[eorder]
    idxi[e_core, e_chunk, e_pos] = slot_of[eorder].astype(np.int16)
    dstl[e_core, e_pos, e_chunk] = dstl_of[eorder].astype(np.float32)

    return dict(S=S, NB=NB, SUB=SUB, nch=nch, cnt_chunks=cnt_chunks,
                base=base, groups=group_info, budget=budget,
                idxj=idxj, idxi=idxi, dstl=dstl)


def _wrap_idx_cols(idx_core):
    """[nch, 128] -> [16, nch*8] (the gather's wrapped idx layout)."""
    nch = idx_core.shape[0]
    a = idx_core.reshape(nch, 8, 16)
    return np.ascontiguousarray(
        np.transpose(a, (2, 0, 1)).reshape(16, nch * 8))


def _chperm(F):
    """original feature index (h-major) for each device feature (c-major)."""
    C = F // HEADS
    f = np.arange(F)
    c, h = f // HEADS, f % HEADS
    return h * C + c


# --------------------------------------------------------------------------
# Device program
# --------------------------------------------------------------------------

def build_program(S, NB, SUB, nch, cnt_chunks, base, groups, budget,
                  layers=3, repeat=1, sim=False, lrelu=False, deep=False):
    KIN = 2                       # IN_C = 256
    HC = 128
    F2 = 256
    layer_F = [HC, HC, F2]

    nc = bacc.Bacc("TRN2", target_bir_lowering=False, debug=False,
                   num_devices=NCORES, num_swdge_queues=4)

    xt_d = [nc.dram_tensor(f"xt{i}", [P, S], BF16, kind="ExternalInput").ap()
            for i in range(KIN)]
    w0_d = [nc.dram_tensor(f"w0_{i}", [P, HC], BF16, kind="ExternalInput").ap()
            for i in range(KIN)]
    w1_d = nc.dram_tensor("w1", [P, HC], BF16, kind="ExternalInput").ap()
    w2_d = nc.dram_tensor("w2", [P, F2], BF16, kind="ExternalInput").ap()
    att_d = [nc.dram_tensor(f"att{i}", [P, layer_F[i]], BF16,
                            kind="ExternalInput").ap() for i in range(3)]
    b01_d = [nc.dram_tensor(f"b{i}", [P, 1], F32, kind="ExternalInput").ap()
             for i in range(2)]
    b2_d = nc.dram_tensor("b2", [64, 1], F32, kind="ExternalInput").ap()
    iota_d = nc.dram_tensor("iota", [P, P], BF16, kind="ExternalInput").ap()
    rmap_d = nc.dram_tensor("rmap", [HEADS, P], F32, kind="ExternalInput").ap()
    msum_d = [nc.dram_tensor(f"msum{i}", [P, 64], BF16,
                             kind="ExternalInput").ap() for i in range(2)]
    idxj_d = nc.dram_tensor("idxj", [16, nch * 8], I16,
                            kind="ExternalInput").ap()
    idxi_d = nc.dram_tensor("idxi", [16, nch * 8], I16,
                            kind="ExternalInput").ap()
    dstl_d = nc.dram_tensor("dstl", [P, nch], BF16, kind="ExternalInput").ap()
    out_d = nc.dram_tensor("out", [S, 64], F32, kind="ExternalOutput").ap()

    with tile.TileContext(nc) as tc:
        with tc.tile_pool(name="const", bufs=1) as cp, \
             tc.tile_pool(name="pers", bufs=1) as pp, \
             tc.tile_pool(name="stageA", bufs=3) as wp, \
             tc.tile_pool(name="slabj", bufs=2) as xjp, \
             tc.tile_pool(name="slabi", bufs=2) as xip, \
             tc.tile_pool(name="smalls", bufs=3 if deep else 2) as sp, \
             tc.tile_pool(name="pm", bufs=5 if deep else 3) as zp, \
             tc.tile_pool(name="tails", bufs=3 if deep else 2) as tp, \
             tc.tile_pool(name="psA", bufs=1 if deep else 2,
                          space="PSUM") as psA, \
             tc.tile_pool(name="psO", bufs=2, space="PSUM") as psO, \
             tc.tile_pool(name="psD", bufs=2 if deep else 1,
                          space="PSUM") as psD, \
             tc.tile_pool(name="psM", bufs=1, space="PSUM") as psM, \
             tc.tile_pool(name="dram", bufs=1, space="DRAM") as dp:

            # ---- constants ----
            w0_sb = [cp.tile([P, HC], BF16, tag=f"w0_{i}", name=f"w0s{i}")
                     for i in range(KIN)]
            for i in range(KIN):
                nc.sync.dma_start(out=w0_sb[i][:], in_=w0_d[i][:])
            w1_sb = cp.tile([P, HC], BF16, tag="w1")
            nc.sync.dma_start(out=w1_sb[:], in_=w1_d[:])
            w2_sb = cp.tile([P, F2], BF16, tag="w2")
            nc.sync.dma_start(out=w2_sb[:], in_=w2_d[:])
            att_sb = []
            for i in range(3):
                t = cp.tile([P, layer_F[i]], BF16, tag=f"att{i}",
                            name=f"atts{i}")
                nc.sync.dma_start(out=t[:], in_=att_d[i][:])
                att_sb.append(t)
            b01_sb = []
            for i in range(2):
                t = cp.tile([P, 1], F32, tag=f"b{i}", name=f"bs{i}")
                nc.sync.dma_start(out=t[:], in_=b01_d[i][:])
                b01_sb.append(t)
            b2_sb = cp.tile([64, 1], F32, tag="b2")
            nc.sync.dma_start(out=b2_sb[:], in_=b2_d[:])
            iota_sb = cp.tile([P, P], BF16, tag="iota")
            nc.sync.dma_start(out=iota_sb[:], in_=iota_d[:])
            rmap_sb = cp.tile([HEADS, P], F32, tag="rmap")
            nc.sync.dma_start(out=rmap_sb[:], in_=rmap_d[:])
            msum_sb = []
            for i in range(2):
                t = cp.tile([P, 64], BF16, tag=f"msum{i}", name=f"msums{i}")
                nc.sync.dma_start(out=t[:], in_=msum_d[i][:])
                msum_sb.append(t)
            dstl_bf = cp.tile([P, nch], BF16, tag="dstlb")
            nc.sync.dma_start(out=dstl_bf[:], in_=dstl_d[:])
            dstl_sb = cp.tile([P, nch], F32, tag="dstl")
            nc.scalar.copy(dstl_sb[:], dstl_bf[:])
            ident = cp.tile([P, P], BF16, tag="ident")
            make_identity(nc, ident[:])
            identf = cp.tile([P, P], F32, tag="identf")
            make_identity(nc, identf[:])

            # ---- persistent ----
            x0T_sb = pp.tile([P, S], BF16, tag="x0T")
            hT_sb = pp.tile([P, S], BF16, tag="hT")

            xl_own = [dp.tile([S, layer_F[l]], BF16, tag=f"xlo{l}",
                              name=f"xlo{l}") for l in range(3)]
            xl_full = [dp.tile([NCORES * S, layer_F[l]], BF16, tag=f"xlf{l}",
                               name=f"xlf{l}") for l in range(3)]
            idxj_rep = dp.tile([P, nch * 8], I16, tag="ijr", name="ijr")
            idxi_rep = dp.tile([P, nch * 8], I16, tag="iir", name="iir")
            for b in range(8):
                nc.sync.dma_start(out=idxj_rep[16 * b:16 * (b + 1), :],
                                  in_=idxj_d[:, :])
                nc.sync.dma_start(out=idxi_rep[16 * b:16 * (b + 1), :],
                                  in_=idxi_d[:, :])

            for l in [ll for _ in range(repeat) for ll in range(layers)]:
                F = layer_F[l]
                C = F // HEADS
                NFS = F // P

                # ---------- phase A:  xl = h @ W ----------
                for k in range(NB):
                    ps = psA.tile([P, F], F32, tag="psA")
                    if l == 0:
                        for i in range(KIN):
                            xa = wp.tile([P, P], BF16, tag=f"xta{i}",
                                         name=f"xta{i}")
                            nc.sync.dma_start(
                                out=xa[:], in_=xt_d[i][:, k * P:(k + 1) * P])
                            nc.tensor.matmul(ps[:], lhsT=xa[:],
                                             rhs=w0_sb[i][:],
                                             start=(i == 0),
                                             stop=(i == KIN - 1))
                    else:
                        w_sb = w1_sb if l == 1 else w2_sb
                        nc.tensor.matmul(ps[:],
                                         lhsT=hT_sb[:, k * P:(k + 1) * P],
                                         rhs=w_sb[:], start=True, stop=True)
                    stage = wp.tile([P, F], BF16, tag="stage")
                    nc.scalar.copy(stage[:], ps[:])
                    nc.sync.dma_start(out=xl_own[l][k * P:(k + 1) * P, :],
                                      in_=stage[:])
                    if l == 0:
                        ptx = psM.tile([P, P], BF16, tag="psM", name="ptx")
                        nc.tensor.transpose(ptx[:], stage[:], ident[:])
                        nc.vector.tensor_scalar(
                            out=x0T_sb[:, k * P:(k + 1) * P], in0=ptx[:],
                            scalar1=RES_ALPHA, scalar2=None, op0=AL.mult)

                # ---------- AllGather ----------
                if sim:
                    for c in range(NCORES):
                        nc.sync.dma_start(
                            out=xl_full[l][c * S:(c + 1) * S, :],
                            in_=xl_own[l][:, :])
                else:
                    nc.gpsimd.collective_compute(
                        "AllGather", AL.bypass,
                        replica_groups=[list(range(NCORES))],
                        ins=[xl_own[l].opt()],
                        outs=[xl_full[l].opt()],
                    )

                # ---------- phase C: edge groups ----------
                for gi in groups:
                    g0 = gi["ch0"]
                    G = gi["nch"]
                    ij = sp.tile([P, budget * 8], I16, tag="ij")
                    ii = sp.tile([P, budget * 8], I16, tag="ii")
                    nc.sync.dma_start(out=ij[:, :G * 8],
                                      in_=idxj_rep[:, g0 * 8:(g0 + G) * 8])
                    nc.sync.dma_start(out=ii[:, :G * 8],
                                      in_=idxi_rep[:, g0 * 8:(g0 + G) * 8])
                    xj = xjp.tile([P, budget * F], BF16, tag="xj")
                    xi = xip.tile([P, budget * F], BF16, tag="xi")
                    for (t, ch0_t, nch_t) in gi["calls"]:
                        sc0 = ch0_t - g0
                        nc.gpsimd.dma_gather(
                            out_ap=xj[:, sc0 * F:(sc0 + nch_t) * F]
                            .rearrange("p (c f) -> p c f", f=F),
                            in_ap=xl_full[l][t * SUB:(t + 1) * SUB, :],
                            idxs_ap=ij[:, sc0 * 8:(sc0 + nch_t) * 8],
                            num_idxs=nch_t * P, num_idxs_reg=nch_t * P,
                            elem_size=F, single_packet=False,
                            queue_num=t % 4)
                    nc.gpsimd.dma_gather(
                        out_ap=xi[:, :G * F].rearrange("p (c f) -> p c f",
                                                       f=F),
                        in_ap=xl_own[l][:, :],
                        idxs_ap=ii[:, :G * 8],
                        num_idxs=G * P, num_idxs_reg=G * P,
                        elem_size=F, single_packet=False,
                        queue_num=1)

                    # slab ops
                    ea = xi[:, :G * F]
                    nc.vector.tensor_tensor(out=ea, in0=ea,
                                            in1=xj[:, :G * F], op=AL.add)
                    if lrelu:
                        nc.scalar.activation(ea, ea, AF.Lrelu,
                                             alpha=NEG_SLOPE)
                    else:
                        nc.vector.scalar_tensor_tensor(
                            out=ea, in0=ea, scalar=NEG_SLOPE, in1=ea,
                            op0=AL.mult, op1=AL.max)
                    nc.vector.tensor_tensor(
                        out=ea.rearrange("p (g f) -> p g f", f=F),
                        in0=ea.rearrange("p (g f) -> p g f", f=F),
                        in1=att_sb[l][:].unsqueeze(1).broadcast_to([P, G, F]),
                        op=AL.mult)
                    # (c,h) layout: alpha[p,(g,h)] = sum_c ea[p,g,c,h]
                    # binary tree-sum over c: packed bf16 adds run at the
                    # DVE 2x rate vs 1x for tensor_reduce
                    ea4 = ea.rearrange("p (g c h) -> p g c h", h=HEADS, c=C)
                    w = C // 2
                    while w > 1:
                        nc.vector.tensor_tensor(
                            out=ea4[:, :, 0:w, :], in0=ea4[:, :, 0:w, :],
                            in1=ea4[:, :, w:2 * w, :], op=AL.add)
                        w //= 2
                    al = sp.tile([P, budget * HEADS], BF16, tag="al")
                    nc.vector.tensor_tensor(
                        out=al[:, :G * HEADS].rearrange("p (g h) -> p g h",
                                                        h=HEADS),
                        in0=ea4[:, :, 0, :], in1=ea4[:, :, 1, :], op=AL.add)
                    ex = sp.tile([P, budget * HEADS], BF16, tag="ex")
                    nc.scalar.activation(ex[:, :G * HEADS], al[:, :G * HEADS],
                                         AF.Exp)
                    nc.vector.tensor_tensor(
                        out=xj[:, :G * F].rearrange("p (g c h) -> p g c h",
                                                    h=HEADS, c=C),
                        in0=xj[:, :G * F].rearrange("p (g c h) -> p g c h",
                                                    h=HEADS, c=C),
                        in1=ex[:, :G * HEADS]
                            .rearrange("p (g h) -> p g h", h=HEADS)
                            .unsqueeze(2).broadcast_to([P, G, C, HEADS]),
                        op=AL.mult)

                    # scatter + per-tile tails
                    for k in gi["tiles"]:
                        po = [psO.tile([P, P], F32, tag=f"psO{fs}",
                                       name=f"psO{fs}") for fs in range(NFS)]
                        pden = psD.tile([HEADS, P], F32, tag="psD",
                                        name="pden")
                        tile_chunks = []
                        for t in range(NSUB):
                            for j in range(int(cnt_chunks[k, t])):
                                tile_chunks.append(int(base[k, t]) + j)
                        for ci, ch in enumerate(tile_chunks):
                            first = ci == 0
                            last = ci == len(tile_chunks) - 1
                            sc = ch - g0
                            Pm = zp.tile([P, P], BF16, tag="Pm")
                            nc.vector.tensor_scalar(
                                out=Pm[:], in0=iota_sb[:],
                                scalar1=dstl_sb[:, ch:ch + 1], scalar2=None,
                                op0=AL.is_equal)
                            for fs in range(NFS):
                                nc.tensor.matmul(
                                    po[fs][:],
                                    lhsT=xj[:, sc * F + fs * P:
                                            sc * F + (fs + 1) * P],
                                    rhs=Pm[:], start=first, stop=last)
                            nc.tensor.matmul(
                                pden[:],
                                lhsT=ex[:, sc * HEADS:(sc + 1) * HEADS],
                                rhs=Pm[:], start=first, stop=last)

                        # ---- tile tail ----
                        rec = tp.tile([HEADS, P], F32, tag="rec")
                        nc.vector.tensor_scalar(out=rec[:], in0=pden[:],
                                                scalar1=EPS, scalar2=None,
                                                op0=AL.add)
                        nc.vector.reciprocal(rec[:], rec[:])
                        scale = (1.0 - RES_ALPHA) if l < 2 else (1.0 / HEADS)
                        nc.vector.tensor_scalar(out=rec[:], in0=rec[:],
                                                scalar1=scale, scalar2=None,
                                                op0=AL.mult)
                        cols = slice(k * P, (k + 1) * P)
                        if l < 2:
                            prep = psM.tile([P, P], F32, tag="psM",
                                            name="prep")
                            nc.tensor.matmul(prep[:], lhsT=rmap_sb[:],
                                             rhs=rec[:], start=True,
                                             stop=True)
                            rep = tp.tile([P, P], F32, tag="rep")
                            nc.scalar.copy(rep[:], prep[:])
                            u = tp.tile([P, P], F32, tag="u")
                            nc.vector.tensor_tensor(out=u[:], in0=po[0][:],
                                                    in1=rep[:], op=AL.mult)
                            nc.scalar.activation(u[:], u[:], AF.Identity,
                                                 bias=b01_sb[l][:, 0:1])
                            nc.vector.tensor_tensor(out=u[:], in0=u[:],
                                                    in1=x0T_sb[:, cols],
                                                    op=AL.add)
                            mn = tp.tile([P, P], F32, tag="mn")
                            nc.vector.tensor_scalar(out=mn[:], in0=u[:],
                                                    scalar1=0.0,
                                                    scalar2=None, op0=AL.min)
                            em = tp.tile([P, P], F32, tag="em")
                            nc.scalar.activation(em[:], mn[:], AF.Exp)
                            hh = tp.tile([P, P], F32, tag="hh")
                            nc.vector.scalar_tensor_tensor(
                                out=hh[:], in0=u[:], scalar=0.0, in1=em[:],
                                op0=AL.max, op1=AL.add)
                            nc.vector.tensor_scalar(
                                out=hT_sb[:, cols], in0=hh[:],
                                scalar1=-1.0, scalar2=None, op0=AL.add)
                        else:
                            tsb = []
                            for fs in range(2):
                                prep = psM.tile([P, P], F32, tag="psM",
                                                name="prep")
                                nc.tensor.matmul(prep[:], lhsT=rmap_sb[:],
                                                 rhs=rec[:], start=True,
                                                 stop=True)
                                rep = tp.tile([P, P], F32, tag="rep")
                                nc.scalar.copy(rep[:], prep[:])
                                tt = tp.tile([P, P], BF16, tag=f"t{fs}",
                                             name=f"tsb{fs}")
                                nc.vector.tensor_tensor(out=tt[:],
                                                        in0=po[fs][:],
                                                        in1=rep[:],
                                                        op=AL.mult)
                                tsb.append(tt)
                            pmo = psM.tile([64, P], F32, tag="psM",
                                           name="pmo")
                            nc.tensor.matmul(pmo[:], lhsT=msum_sb[0][:],
                                             rhs=tsb[0][:], start=True,
                                             stop=False)
                            nc.tensor.matmul(pmo[:], lhsT=msum_sb[1][:],
                                             rhs=tsb[1][:], start=False,
                                             stop=True)
                            ob = tp.tile([64, P], F32, tag="ob")
                            nc.scalar.activation(ob[:], pmo[:], AF.Identity,
                                                 bias=b2_sb[:, 0:1])
                            pot = psM.tile([P, 64], F32, tag="psM",
                                           name="pot")
                            nc.tensor.transpose(pot[:], ob[:],
                                                identf[:64, :64])
                            orow = tp.tile([P, 64], F32, tag="orow")
                            nc.scalar.copy(orow[:], pot[:])
                            nc.sync.dma_start(
                                out=out_d[k * P:(k + 1) * P, :],
                                in_=orow[:])

    nc.compile()
    return nc


# --------------------------------------------------------------------------
# kernel() entry
# --------------------------------------------------------------------------

def prepare(x, edge_index, W0, b0, att0, W1, b1, att1, W2, b2, att2,
            _budget=40, _layers=3, _repeat=1, _sim=False, _lrelu=False,
            _deep=False):
    x = np.asarray(x, dtype=np.float32)
    N, IN_C = x.shape
    pr = _prep(edge_index, N, _budget)
    S, NB, SUB, nch = pr["S"], pr["NB"], pr["SUB"], pr["nch"]

    nc = build_program(S, NB, SUB, nch, pr["cnt_chunks"], pr["base"],
                       pr["groups"], pr["budget"],
                       layers=_layers, repeat=_repeat, sim=_sim,
                       lrelu=_lrelu, deep=_deep)

    KIN = IN_C // P
    p128 = _chperm(128)     # device feature -> original feature (F=128)
    p256 = _chperm(256)

    W0p = np.asarray(W0, np.float32)[:, p128]           # cols -> (c,h)
    W1p = np.asarray(W1, np.float32)[p128][:, p128]     # rows from hT, cols
    W2p = np.asarray(W2, np.float32)[p128][:, p256]

    common = {}
    for i in range(KIN):
        common[f"w0_{i}"] = W0p[i * P:(i + 1) * P, :].astype(nbf)
    common["w1"] = W1p.astype(nbf)
    common["w2"] = W2p.astype(nbf)

    def rep_att(att, F):
        flat = np.asarray(att, np.float32).reshape(-1)[_chperm(F)]
        return np.repeat(flat[None, :], P, axis=0).astype(nbf)

    common["att0"] = rep_att(att0, 128)
    common["att1"] = rep_att(att1, 128)
    common["att2"] = rep_att(att2, 256)
    common["b0"] = ((1.0 - RES_ALPHA) * np.asarray(b0, np.float32)[p128]
                    ).reshape(P, 1)
    common["b1"] = ((1.0 - RES_ALPHA) * np.asarray(b1, np.float32)[p128]
                    ).reshape(P, 1)
    common["b2"] = np.asarray(b2, np.float32).reshape(64, 1)
    common["iota"] = np.tile(np.arange(P, dtype=np.float32)[None, :],
                             (P, 1)).astype(nbf)
    # head of device feature f is f % HEADS (same map for all layers)
    r0 = np.zeros((HEADS, P), np.float32)
    for f in range(P):
        r0[f % HEADS, f] = 1.0
    common["rmap"] = r0
    # l2 head-mean: device feature g=fs*128+f maps to out channel g//HEADS
    for fs in range(2):
        m = np.zeros((P, 64), np.float32)
        for f in range(P):
            m[f, (fs * P + f) // HEADS] = 1.0
        common[f"msum{fs}"] = m.astype(nbf)

    in_maps = []
    for c in range(NCORES):
        m = dict(common)
        nodes = np.arange(c, N, NCORES, dtype=np.int64)
        xc = np.zeros((S, IN_C), dtype=np.float32)
        xc[:len(nodes)] = x[nodes]
        xct = np.ascontiguousarray(xc.T).astype(nbf)
        for i in range(KIN):
            m[f"xt{i}"] = xct[i * P:(i + 1) * P, :].copy()
        m["idxj"] = _wrap_idx_cols(pr["idxj"][c])
        m["idxi"] = _wrap_idx_cols(pr["idxi"][c])
        m["dstl"] = pr["dstl"][c].astype(nbf)
        in_maps.append(m)

    def assemble(per_core_out):
        out = np.zeros((N, 64), dtype=np.float32)
        for c in range(NCORES):
            nodes = np.arange(c, N, NCORES, dtype=np.int64)
            out[nodes] = per_core_out[c][:len(nodes)]
        return out

    return nc, in_maps, assemble


def kernel(x, edge_index, W0, b0, att0, W1, b1, att1, W2, b2, att2, **kw):
    nc, in_maps, assemble = prepare(x, edge_index, W0, b0, att0, W1, b1,
                                    att1, W2, b2, att2, **kw)
    # run twice: the very first execution on freshly-initialized devices has
    # been observed (rarely) to race; the second run is authoritative.
    bass_utils.run_bass_kernel_spmd(nc, in_maps,
                                    core_ids=list(range(NCORES)))
    res = bass_utils.run_bass_kernel_spmd(nc, in_maps,
                                          core_ids=list(range(NCORES)))
    return assemble([res.results[c]["out"] for c in range(NCORES)])

